# revision 33
# baseline (speedup 1.0000x reference)
"""Trainium2 Bass kernel for nn_AttentionGenerator (gnn_message_passing).

Reference math:
    f = einsum('oc,bctv->botv', Wf, feat) + bf          # 1x1 conv, Cout=64
    s_i = einsum('c,bctv->btv', Wa[:64], f)
    s_j = einsum('c,bctv->btv', Wa[64:], f)
    score[b,t,i,j] = s_i[b,t,i] + s_j[b,t,j] + ba
    atten = (exp(leaky_relu(score)) * A) / row_sum

Because f only enters through the two dot products, fold Wf/bf/Wa/ba on
the host into u1 = w1@Wf, u2 = w2@Wf (length-256 vectors) and the scalar
c0 = (w1+w2)@bf + ba.  The device then computes, per (b,t,v), two
channel contractions (TensorEngine), an 18x18 broadcast-add + LeakyReLU
+ exp*A + row-normalize (Vector/Scalar engines).  Memory bound: reads
151 MB of feat, writes 10.6 MB.

Sharding: pure data parallel — batch B=32 split across 8 NeuronCores
(4 batches each), tiny params replicated, no cross-core comms.
"""

import json
import numpy as np
from contextlib import ExitStack

B, Cin, T, V = 32, 256, 256, 18
NCORES = 8
BPC = B // NCORES  # batches per core
TV = T * V
PB = 128  # t-block size (partition dim)
NTB = T // PB

_cached_nc = None


def _legalize_waits_json(bir_json):
    """Split instructions carrying >1 sync wait into single-wait NoOps plus
    the original instruction.  The walrus build in this container accepts at
    most ONE sync-wait command per instruction struct; concourse's Tile
    scheduler freely attaches several.  Hoisting the extra waits onto NoOps
    immediately before the instruction (same engine stream, same position)
    preserves semantics exactly — engines execute their stream in order."""
    bir = json.loads(bir_json)
    ctr = 0
    for fn in bir.get("functions", []):
        for blk in fn.get("blocks", []):
            insts = blk.get("instructions")
            if not insts:
                continue
            out = []
            for inst in insts:
                si = inst.get("sync_info") or {}
                waits = si.get("on_wait") or []
                if len(waits) > 1:
                    for w in waits[:-1]:
                        out.append(
                            {
                                "engine": inst.get("engine"),
                                "ins": [],
                                "name": f"wsplit-{ctr}",
                                "opcode": "NoOp",
                                "outs": [],
                                "sync_info": {"on_update": [], "on_wait": [w]},
                            }
                        )
                        ctr += 1
                    si = dict(si)
                    si["on_wait"] = [waits[-1]]
                    inst = dict(inst)
                    inst["sync_info"] = si
                out.append(inst)
            blk["instructions"] = out
    return json.dumps(bir).encode()


_wait_patch_done = False


def _install_wait_legalizer():
    global _wait_patch_done
    if _wait_patch_done:
        return
    import concourse.bass_utils as bass_utils
    import concourse.bass2jax as bass2jax

    orig = bass_utils.compile_bir_kernel

    def wrapped(bir_json, tmpdir, neff_name="file.neff"):
        return orig(_legalize_waits_json(bir_json), tmpdir, neff_name)

    bass_utils.compile_bir_kernel = wrapped
    bass2jax.compile_bir_kernel = wrapped
    _wait_patch_done = True


def _build_nc():
    import concourse.bass as bass
    import concourse.mybir as mybir
    import concourse.tile as tile
    from concourse.alu_op_type import AluOpType

    f32 = mybir.dt.float32
    # feat/weights move through the PE in bf16: halves the dominant HBM
    # stream (the kernel is memory-bound) and runs the moving operand at
    # 1 cycle/row.  Accumulation stays fp32 in PSUM; quantization costs
    # ~7e-4 relative error on the output.
    bf16 = mybir.dt.bfloat16
    nc = bass.Bass(num_swdge_queues=4)
    feat = nc.dram_tensor("feat", [BPC, Cin, T, V], bf16, kind="ExternalInput")
    # wmat[k, c, o]: o-th contraction vector (u1/u2), c-chunk k of 128
    wmat = nc.dram_tensor("wmat", [2, 128, 2], bf16, kind="ExternalInput")
    amat = nc.dram_tensor("amat", [V, V], f32, kind="ExternalInput")
    cmat = nc.dram_tensor("cmat", [1, 1], f32, kind="ExternalInput")
    out = nc.dram_tensor("out", [BPC, T, V, V], f32, kind="ExternalOutput")

    with ExitStack() as ctx:
        tc = ctx.enter_context(tile.TileContext(nc))
        singles = ctx.enter_context(tc.tile_pool(name="singles", bufs=1))
        fpool = ctx.enter_context(tc.tile_pool(name="fpool", bufs=BPC * NTB))
        pspool = ctx.enter_context(tc.tile_pool(name="pspool", bufs=3, space="PSUM"))
        spool = ctx.enter_context(tc.tile_pool(name="spool", bufs=4))
        work = ctx.enter_context(tc.tile_pool(name="work", bufs=6))
        opool = ctx.enter_context(tc.tile_pool(name="opool", bufs=6))

        w_t = singles.tile([128, 2, 2], bf16)
        nc.sync.dma_start(out=w_t, in_=wmat[:, :, :].rearrange("k p o -> p k o"))
        a_bc = singles.tile([128, V, V], f32)
        nc.sync.dma_start(out=a_bc, in_=amat[:, :].partition_broadcast(128))
        c0_t = singles.tile([128, 1], f32)
        nc.sync.dma_start(out=c0_t, in_=cmat[0, :].partition_broadcast(128))

        # Absorb const-DMA waits on cheap ops so steady-state instructions
        # carry fewer sync waits (less NoOp splitting at compile).
        warm_ps = ps2pool.tile([2, 2], f32, tag="warm")
        nc.tensor.matmul(
            out=warm_ps, lhsT=w_t[:, 0, :], rhs=w_t[:, 0, :], start=True, stop=True
        )
        scratch_c = singles.tile([128, 1], f32)
        nc.vector.tensor_copy(out=scratch_c, in_=c0_t)
        scratch_a = singles.tile([128, V, V], f32)
        nc.vector.tensor_copy(out=scratch_a, in_=a_bc)

        BV = PB * V  # 2304: free size of one t-block

        def transpose_stage(st):
            """SBUF scatter s_sb[o, (t v)] -> [t, v] tiles via the ACT HWDGE
            ring (the SWDGE ring processes these 72B-row descriptors far too
            slowly, and the SP ring carries the feat prefetch stream)."""
            s1t = work.tile([128, V], f32)
            nc.gpsimd.dma_start(
                out=s1t, in_=st["s_sb"][0:1, :].rearrange("o (t v) -> o t v", v=V)
            )
            s2t = work.tile([128, V], f32)
            nc.gpsimd.dma_start(
                out=s2t, in_=st["s_sb"][1:2, :].rearrange("o (t v) -> o t v", v=V)
            )
            st["s1t"], st["s2t"] = s1t, s2t

        def head_stage(st):
            """DVE head (lag 2): score broadcast-add + LeakyReLU."""
            s1t, s2t = st["s1t"], st["s2t"]
            sc = work.tile([128, V, V], f32)
            s1b = bass.AP(
                tensor=s1t.tensor,
                offset=s1t.offset,
                ap=[s1t.ap[0], [1, V], [0, V]],
            )
            s2b = bass.AP(
                tensor=s2t.tensor,
                offset=s2t.offset,
                ap=[s2t.ap[0], [0, V], [1, V]],
            )
            # sc = (s1 + c0) + s2
            nc.vector.scalar_tensor_tensor(
                out=sc,
                in0=s1b,
                scalar=c0_t[:, :],
                in1=s2b,
                op0=AluOpType.add,
                op1=AluOpType.add,
            )
            # LeakyReLU(x) = max(x, 0.1*x)
            lr = work.tile([128, V, V], f32)
            nc.vector.scalar_tensor_tensor(
                out=lr,
                in0=sc,
                scalar=0.1,
                in1=sc,
                op0=AluOpType.mult,
                op1=AluOpType.max,
            )
            st["lr"] = lr

        def exp_stage(st):
            """ACT exp (lag 2)."""
            ex = work.tile([128, V, V], f32)
            nc.scalar.activation(
                out=ex, in_=st["lr"], func=mybir.ActivationFunctionType.Exp
            )
            st["ex"] = ex

        def tail_stage(st):
            """DVE tail (lag 2): exa = ex*A, row-sum, reciprocal, normalize."""
            exa = work.tile([128, V, V], f32)
            nc.vector.tensor_mul(out=exa, in0=st["ex"], in1=a_bc)
            ssum = work.tile([128, V], f32)
            nc.vector.reduce_sum(out=ssum, in_=exa, axis=mybir.AxisListType.X)
            rec = work.tile([128, V], f32)
            nc.vector.reciprocal(out=rec, in_=ssum)
            att = opool.tile([128, V, V], f32)
            rbc = bass.AP(
                tensor=rec.tensor,
                offset=rec.offset,
                ap=[rec.ap[0], [1, V], [0, V]],
            )
            nc.vector.tensor_mul(out=att, in0=exa, in1=rbc)
            st["att"] = att

        def out_stage(st):
            """Output DMA (lag 4) on the ACT HWDGE ring -- the SP ring's
            FIFO carries the feat prefetch stream, and the SWDGE queue is
            kept for the scatters (outs there slow both)."""
            nc.scalar.dma_start(
                out=out[st["b"], st["tb"] * PB : (st["tb"] + 1) * PB],
                in_=st["att"],
            )

        stages = []
        for b in range(BPC):
            for tb in range(NTB):
                n = len(stages)
                # feat[b, :, tb-block, :] as [c_in_chunk, chunk, t*v]
                f_t = fpool.tile([128, 2, BV], bf16)
                nc.sync.dma_start(
                    out=f_t,
                    in_=feat[
                        b, :, tb * PB : (tb + 1) * PB, :
                    ].rearrange("(k p) t v -> p k (t v)", p=128),
                )
                # s[o, t*v] = sum_c u_o[c] * feat[c, t*v]: w chunk stationary
                # (2-column LDW), feat moving in bank-aligned <=512 slices
                # (fp32r runs the moving operand at 1 cycle/row when the
                # slice is >=256), accumulated over the two c-chunks in PSUM.
                s_sb = spool.tile([2, BV], f32)
                ps_small = None
                for si, (base, widths) in enumerate(
                    ((0, (512, 512)), (1024, (512, 512)), (2048, (256,)))
                ):
                    ps = pspool.tile(
                        [2, sum(widths)],
                        f32,
                        tag="ps" if si < 2 else "psc",
                        bufs=None if si < 2 else 2,
                    )
                    lo = 0
                    for w in widths:
                        for k in range(2):
                            nc.tensor.matmul(
                                out=ps[:, lo : lo + w],
                                lhsT=w_t[:, k, :],
                                rhs=f_t[:, k, base + lo : base + lo + w],
                                start=(k == 0),
                                stop=(k == 1),
                            )
                        lo += w
                    if si < 2:
                        nc.scalar.copy(
                            out=s_sb[:, base : base + sum(widths)], in_=ps
                        )
                    else:
                        ps_small = ps
                stages.append(
                    {"s_sb": s_sb, "ps_small": ps_small, "b": b, "tb": tb}
                )
                # lag-1: drain the small PSUM slice on DVE one iteration
                # later -- PE(n) is long done, so the in-order DVE queue
                # never stalls on the PE wait
                if n >= 1:
                    pst = stages[n - 1]
                    nc.vector.tensor_copy(
                        out=pst["s_sb"][:, 2048:BV], in_=pst["ps_small"]
                    )
                if n >= 2:
                    transpose_stage(stages[n - 2])
                    head_stage(stages[n - 2])
                if n >= 3:
                    exp_stage(stages[n - 3])
                    tail_stage(stages[n - 3])
                if n >= 4:
                    out_stage(stages[n - 4])
        nc.vector.tensor_copy(
            out=stages[-1]["s_sb"][:, 2048:BV], in_=stages[-1]["ps_small"]
        )
        for st in stages[-2:]:
            transpose_stage(st)
            head_stage(st)
        for st in stages[-3:]:
            exp_stage(st)
            tail_stage(st)
        for st in stages[-4:]:
            out_stage(st)
    return nc


def _prep_params(Wf, bf, Wa, ba):
    import ml_dtypes  # noqa: F401
    w1, w2 = Wa[:64].astype(np.float64), Wa[64:].astype(np.float64)
    Wf64, bf64 = Wf.astype(np.float64), bf.astype(np.float64)
    u1 = w1 @ Wf64
    u2 = w2 @ Wf64
    c0 = float(w1 @ bf64 + w2 @ bf64 + float(ba[0]))
    import ml_dtypes

    wmat = np.stack([u1, u2], axis=-1).reshape(2, 128, 2).astype(ml_dtypes.bfloat16)
    cmat = np.full((1, 1), c0, dtype=np.float32)
    return wmat, cmat


def get_nc():
    global _cached_nc
    if _cached_nc is None:
        _cached_nc = _build_nc()
    return _cached_nc


def kernel(feat, A, Wf, bf, Wa, ba):
    _install_wait_legalizer()
    from concourse.bass_utils import run_bass_kernel_spmd

    import ml_dtypes

    feat = np.ascontiguousarray(np.asarray(feat, dtype=np.float32).astype(ml_dtypes.bfloat16))
    A = np.ascontiguousarray(np.asarray(A, dtype=np.float32))
    wmat, cmat = _prep_params(
        np.asarray(Wf, np.float32),
        np.asarray(bf, np.float32),
        np.asarray(Wa, np.float32),
        np.asarray(ba, np.float32),
    )

    nc = get_nc()
    in_maps = [
        {
            "feat": feat[i * BPC : (i + 1) * BPC],
            "wmat": wmat,
            "amat": A,
            "cmat": cmat,
        }
        for i in range(NCORES)
    ]
    res = run_bass_kernel_spmd(nc, in_maps, core_ids=list(range(NCORES)))
    return np.concatenate([r["out"] for r in res.results], axis=0)
